# revision 1
# baseline (speedup 1.0000x reference)
"""HGNN conv on 8 trn2 cores.

out = D_v^-1 H D_e^-1 H^T input W + bias   (W commutes past the aggregations)

Phase A (edge-sharded): cores own contiguous 3200-edge ranges. Entries sorted
by E; per 128-edge window, entry tiles are gathered (indirect DMA on V) and
scatter-accumulated into PSUM via one-hot matmul; window * recip_e -> y_shard.
AllGather y_shard -> y_full [25600,128] on every core.
Phase B (node-sharded): cores own contiguous 6272-node ranges. Entries sorted
by V; per 128-node window, gather y_full rows by E, one-hot matmul with
swapped operands accumulates Z^T; then Z@W (* recip_v) + bias -> out rows.
"""
import os
import sys

for _p in ('/opt/trn_rl_repo', '/root/.axon_site/_ro/trn_rl_repo'):
    if os.path.isdir(_p) and _p not in sys.path:
        sys.path.insert(0, _p)

import numpy as np

P = 128
NCORES = 8
N_NODE = 50000
N_EDGE = 25000
D = 128
ESH = 3200            # edges per core shard (8*3200 = 25600 >= 25000)
NSH = 6272            # nodes per core shard (49*128; 8*6272 = 50176 >= 50000)
W_A = ESH // P        # 25 edge windows per core
W_B = NSH // P        # 49 node windows per core
NCH = 5               # allgather chunks (5 windows = 640 edges each)
CH_E = ESH // NCH     # 640 edges per chunk per core


def _row_of_edge(e):
    """y_full row for global edge id, chunk-major allgather layout."""
    c = e // ESH
    k = (e % ESH) // CH_E
    return k * (NCORES * CH_E) + c * CH_E + e % CH_E

_PROG_CACHE = {}
LAST_RESULTS = None


def _pack_windows(sorted_idx, sorted_slotbase, bnd, n_shards, n_win, F):
    """sorted_idx: gather row per entry; bnd: window entry boundaries."""
    vidx = np.zeros((n_shards, n_win, P, F), np.int32)
    slot = np.full((n_shards, n_win, P, F), -1.0, np.float32)
    for c in range(n_shards):
        for w in range(n_win):
            wi = c * n_win + w
            lo, hi = bnd[wi], bnd[wi + 1]
            n = hi - lo
            if n == 0:
                continue
            vv = np.zeros(F * P, np.int32)
            vv[:n] = sorted_idx[lo:hi]
            ss = np.full(F * P, -1.0, np.float32)
            ss[:n] = sorted_slotbase[lo:hi] - wi * P
            vidx[c, w] = vv.reshape(F, P).T
            slot[c, w] = ss.reshape(F, P).T
    return vidx, slot


def _preprocess(V, E):
    V = np.asarray(V).astype(np.int64)
    E = np.asarray(E).astype(np.int64)

    oA = np.argsort(E, kind='stable')
    Es, Vs = E[oA], V[oA]
    bndA = np.searchsorted(Es, np.arange(0, NCORES * ESH + 1, P))
    FA = int(np.ceil(np.diff(bndA).max() / P))
    a_vidx, a_slot = _pack_windows(Vs, Es, bndA, NCORES, W_A, FA)
    cntE = np.bincount(E, minlength=N_EDGE).astype(np.float64)
    recipE = (1.0 / np.maximum(cntE, 1.0)).astype(np.float32)
    er = np.arange(NCORES * ESH)
    a_recip = np.where(er < N_EDGE, recipE[np.minimum(er, N_EDGE - 1)],
                       0.0).astype(np.float32).reshape(NCORES, W_A, P)

    oB = np.argsort(V, kind='stable')
    Vs2, Es2 = V[oB], E[oB]
    bndB = np.searchsorted(Vs2, np.arange(0, NCORES * NSH + 1, P))
    FB = int(np.ceil(np.diff(bndB).max() / P))
    b_eidx, b_slot = _pack_windows(_row_of_edge(Es2), Vs2, bndB,
                                   NCORES, W_B, FB)
    cntV = np.bincount(V, minlength=N_NODE).astype(np.float64)
    recipV = (1.0 / np.maximum(cntV, 1.0)).astype(np.float32)
    nr = np.arange(NCORES * NSH)
    b_recip = np.where(nr < N_NODE, recipV[np.minimum(nr, N_NODE - 1)],
                       0.0).astype(np.float32).reshape(NCORES, W_B, P)

    return dict(FA=FA, FB=FB, a_vidx=a_vidx, a_slot=a_slot, a_recip=a_recip,
                b_eidx=b_eidx, b_slot=b_slot, b_recip=b_recip)


def _emulate(pp, inp_f32, weight, bias):
    """Numpy emulation of the exact device program (for logic validation)."""
    FA, FB = pp['FA'], pp['FB']
    iota = np.arange(P, dtype=np.float32)
    y_full = np.zeros((NCORES * ESH, D), np.float32)
    for c in range(NCORES):
        for w in range(W_A):
            acc = np.zeros((P, D), np.float32)
            for f in range(FA):
                g = inp_f32[pp['a_vidx'][c, w, :, f]]
                o = (iota[None, :] == pp['a_slot'][c, w, :, f][:, None])
                acc += o.astype(np.float32).T @ g
            r0 = _row_of_edge(c * ESH + w * P)
            y_full[r0:r0 + P] = acc * pp['a_recip'][c, w][:, None]
    out = np.zeros((NCORES * NSH, D), np.float32)
    for c in range(NCORES):
        for w in range(W_B):
            acc2 = np.zeros((D, P), np.float32)
            for f in range(FB):
                g = y_full[pp['b_eidx'][c, w, :, f]]
                o = (iota[None, :] == pp['b_slot'][c, w, :, f][:, None])
                acc2 += g.T @ o.astype(np.float32)
            z = acc2.T
            res = (z @ weight) * pp['b_recip'][c, w][:, None] + bias[None, :]
            out[(c * W_B + w) * P + c * (NSH - W_B * P):][:P] = res  # NSH==W_B*P
    rows = []
    for c in range(NCORES):
        n = min(NSH, N_NODE - c * NSH)
        rows.append(out[c * NSH:c * NSH + n])
    return np.concatenate(rows, 0)


def _build_program(FA, FB):
    import concourse.bacc as bacc
    import concourse.bass as bass
    import concourse.tile as tile
    from concourse import mybir

    f32 = mybir.dt.float32
    bf16 = mybir.dt.bfloat16
    i32 = mybir.dt.int32

    nc = bacc.Bacc(None, target_bir_lowering=False, debug=False)
    inp = nc.dram_tensor("input", [N_NODE, D], bf16, kind="ExternalInput")
    wgt = nc.dram_tensor("wgt", [D, D], bf16, kind="ExternalInput")
    bias_bc = nc.dram_tensor("bias_bc", [P, D], f32, kind="ExternalInput")
    iota_in = nc.dram_tensor("iota_in", [P, P], f32, kind="ExternalInput")
    a_vidx = nc.dram_tensor("a_vidx", [W_A, P, FA], i32, kind="ExternalInput")
    a_slot = nc.dram_tensor("a_slot", [W_A, P, FA], f32, kind="ExternalInput")
    a_recip = nc.dram_tensor("a_recip", [W_A, P], f32, kind="ExternalInput")
    b_eidx = nc.dram_tensor("b_eidx", [W_B, P, FB], i32, kind="ExternalInput")
    b_slot = nc.dram_tensor("b_slot", [W_B, P, FB], f32, kind="ExternalInput")
    b_recip = nc.dram_tensor("b_recip", [W_B, P], f32, kind="ExternalInput")
    out = nc.dram_tensor("out", [NSH, D], f32, kind="ExternalOutput")

    eq = mybir.AluOpType.is_equal
    mul = mybir.AluOpType.mult
    add = mybir.AluOpType.add

    with tile.TileContext(nc) as tc:
        with tc.tile_pool(name="const", bufs=1) as cpool, \
             tc.tile_pool(name="idx", bufs=3) as ipool, \
             tc.tile_pool(name="slt", bufs=3) as spool, \
             tc.tile_pool(name="rcp", bufs=3) as rpool, \
             tc.tile_pool(name="gat", bufs=12) as gpool, \
             tc.tile_pool(name="one", bufs=12) as opool, \
             tc.tile_pool(name="res", bufs=4) as respool, \
             tc.tile_pool(name="pacc", bufs=2, space="PSUM") as pacc, \
             tc.tile_pool(name="pres", bufs=2, space="PSUM") as pres, \
             tc.tile_pool(name="dram", bufs=1, space="DRAM") as dpool:

            iota_t = cpool.tile([P, P], f32)
            nc.sync.dma_start(out=iota_t[:], in_=iota_in[:])
            wgt_t = cpool.tile([D, D], bf16)
            nc.sync.dma_start(out=wgt_t[:], in_=wgt[:])
            bias_t = cpool.tile([P, D], f32)
            nc.sync.dma_start(out=bias_t[:], in_=bias_bc[:])

            y_shard = dpool.tile([ESH, D], bf16)
            y_full = dpool.tile([NCORES * ESH, D], bf16)
            y_ch = [dpool.tile([NCORES * CH_E, D], bf16, addr_space="Shared",
                               name=f"y_ch{k}") for k in range(NCH)]

            # ---------------- Phase A ----------------
            for w in range(W_A):
                vidx_t = ipool.tile([P, FA], i32)
                nc.sync.dma_start(out=vidx_t[:], in_=a_vidx[w])
                slot_t = spool.tile([P, FA], f32)
                nc.sync.dma_start(out=slot_t[:], in_=a_slot[w])
                recip_t = rpool.tile([P, 1], f32)
                nc.sync.dma_start(out=recip_t[:], in_=a_recip[w, :, None])
                acc = pacc.tile([P, D], f32)
                for f in range(FA):
                    g = gpool.tile([P, D], bf16, name="g", tag="g_a")
                    nc.gpsimd.indirect_dma_start(
                        out=g[:], out_offset=None, in_=inp[:],
                        in_offset=bass.IndirectOffsetOnAxis(
                            ap=vidx_t[:, f:f + 1], axis=0))
                    o = opool.tile([P, P], bf16)
                    nc.vector.tensor_tensor(
                        out=o[:], in0=iota_t[:],
                        in1=slot_t[:, f:f + 1].to_broadcast([P, P]), op=eq)
                    nc.tensor.matmul(acc[:], lhsT=o[:], rhs=g[:],
                                     start=(f == 0), stop=(f == FA - 1))
                yw = respool.tile([P, D], bf16, name="yw", tag="yw")
                nc.vector.tensor_tensor(
                    out=yw[:], in0=acc[:],
                    in1=recip_t[:, :1].to_broadcast([P, D]), op=mul)
                nc.sync.dma_start(out=y_shard[w * P:(w + 1) * P, :], in_=yw[:])
                if (w + 1) % (W_A // NCH) == 0:
                    k = w // (W_A // NCH)
                    nc.gpsimd.collective_compute(
                        "AllGather", mybir.AluOpType.bypass,
                        replica_groups=[list(range(NCORES))],
                        ins=[y_shard[k * CH_E:(k + 1) * CH_E, :]],
                        outs=[y_ch[k].opt()])
                    nc.sync.dma_start(
                        out=y_full[k * NCORES * CH_E:
                                   (k + 1) * NCORES * CH_E, :],
                        in_=y_ch[k][:])

            # ---------------- Phase B ----------------
            for w in range(W_B):
                eidx_t = ipool.tile([P, FB], i32, name="eidx_t", tag="idx_b")
                nc.sync.dma_start(out=eidx_t[:], in_=b_eidx[w])
                slot_t = spool.tile([P, FB], f32, name="slot_tb", tag="slt_b")
                nc.sync.dma_start(out=slot_t[:], in_=b_slot[w])
                recip_t = rpool.tile([P, 1], f32, name="recip_tb")
                nc.sync.dma_start(out=recip_t[:], in_=b_recip[w, :, None])
                acc2 = pacc.tile([P, D], f32, name="acc2")
                for f in range(FB):
                    g = gpool.tile([P, D], bf16, name="gb", tag="g_b")
                    nc.gpsimd.indirect_dma_start(
                        out=g[:], out_offset=None, in_=y_full[:],
                        in_offset=bass.IndirectOffsetOnAxis(
                            ap=eidx_t[:, f:f + 1], axis=0))
                    o = opool.tile([P, P], bf16, name="ob")
                    nc.vector.tensor_tensor(
                        out=o[:], in0=iota_t[:],
                        in1=slot_t[:, f:f + 1].to_broadcast([P, P]), op=eq)
                    nc.tensor.matmul(acc2[:], lhsT=g[:], rhs=o[:],
                                     start=(f == 0), stop=(f == FB - 1))
                zt = respool.tile([P, D], bf16, name="zt", tag="zt")
                nc.vector.tensor_copy(out=zt[:], in_=acc2[:])
                res_p = pres.tile([P, D], f32, name="res_p")
                nc.tensor.matmul(res_p[:], lhsT=zt[:], rhs=wgt_t[:],
                                 start=True, stop=True)
                tmp = respool.tile([P, D], f32, name="tmpb")
                nc.vector.tensor_tensor(
                    out=tmp[:], in0=res_p[:],
                    in1=recip_t[:, :1].to_broadcast([P, D]), op=mul)
                res = respool.tile([P, D], f32, name="resb")
                nc.vector.tensor_tensor(out=res[:], in0=tmp[:], in1=bias_t[:],
                                        op=add)
                nc.sync.dma_start(out=out[w * P:(w + 1) * P, :], in_=res[:])

    nc.compile()
    return nc


def kernel(input, weight, bias, V, E, num_edges):
    global LAST_RESULTS
    inp = np.ascontiguousarray(np.asarray(input), dtype=np.float32)
    wgt = np.ascontiguousarray(np.asarray(weight), dtype=np.float32)
    b = np.asarray(bias).astype(np.float32)
    pp = _preprocess(V, E)

    if os.environ.get('KERNEL_EMULATE'):
        return _emulate(pp, inp, wgt, b)

    from concourse.bass_utils import run_bass_kernel_spmd

    key = (pp['FA'], pp['FB'])
    if key not in _PROG_CACHE:
        _PROG_CACHE[key] = _build_program(*key)
    nc = _PROG_CACHE[key]

    import ml_dtypes
    bf = ml_dtypes.bfloat16
    iota_np = np.tile(np.arange(P, dtype=np.float32), (P, 1))
    bias_bc = np.tile(b[None, :], (P, 1)).astype(np.float32)
    in_maps = []
    for c in range(NCORES):
        in_maps.append(dict(
            input=inp.astype(bf), wgt=wgt.astype(bf),
            bias_bc=bias_bc, iota_in=iota_np,
            a_vidx=pp['a_vidx'][c], a_slot=pp['a_slot'][c],
            a_recip=pp['a_recip'][c],
            b_eidx=pp['b_eidx'][c], b_slot=pp['b_slot'][c],
            b_recip=pp['b_recip'][c]))

    trace = bool(os.environ.get('KERNEL_TRACE'))
    res = run_bass_kernel_spmd(nc, in_maps, list(range(NCORES)), trace=trace)
    LAST_RESULTS = res
    rows = []
    for c in range(NCORES):
        n = min(NSH, N_NODE - c * NSH)
        rows.append(res.results[c]['out'][:n])
    return np.concatenate(rows, 0).astype(np.float32)



# revision 2
# speedup vs baseline: 2.2562x; 2.2562x over previous
"""HGNN conv on 8 trn2 cores — v2: dma_gather-based.

out = D_v^-1 H D_e^-1 H^T input W + bias   (W commutes past the aggregations)

Phase A (edge-sharded): cores own contiguous 3200-edge ranges; entries sorted
by E. Per 128-edge window, entry rows are gathered with chunked dma_gather
(512 rows/instr, 4 swdge queues). int16 gather indices can't span 50000 rows,
so each window's entries are split into lo (V<32768) / hi slabs gathered from
offset views of the table. One batched DVE is_equal generates the window's
one-hot [128, F, 128]; F matmuls scatter-accumulate into PSUM; * recip_e ->
y_shard. AllGather in 5 chunks -> y_full [25600,128] on every core.
Phase B (node-sharded): same machinery against y_full (25600 rows fits int16
directly); matmuls with swapped operands accumulate Z^T; Z@W * recip_v + bias.
Per-window slice counts are padded to the max across cores so all 8 cores run
one SPMD program.
"""
import os
import sys

for _p in ('/opt/trn_rl_repo', '/root/.axon_site/_ro/trn_rl_repo'):
    if os.path.isdir(_p) and _p not in sys.path:
        sys.path.insert(0, _p)

import numpy as np

P = 128
NCORES = 8
N_NODE = 50000
N_EDGE = 25000
D = 128
ESH = 3200            # edges per core shard
NSH = 6272            # nodes per core shard (49*128)
W_A = ESH // P        # 25 edge windows per core
W_B = NSH // P        # 49 node windows per core
NCH = 5               # allgather chunks (5 windows = 640 edges each)
CH_E = ESH // NCH     # 640 edges per chunk per core
LO = 32768            # int16 index limit for phase A lo/hi split
CHR = int(os.environ.get('KERNEL_CHR', '512'))  # gather rows per instr
NQ = int(os.environ.get('KERNEL_NQ', '4'))  # swdge queues


def _row_of_edge(e):
    """y_full row for global edge id, chunk-major allgather layout."""
    c = e // ESH
    k = (e % ESH) // CH_E
    return k * (NCORES * CH_E) + c * CH_E + e % CH_E


_PROG_CACHE = {}
LAST_RESULTS = None


def _pack_seq(idx, slot, F):
    """idx/slot for one padded segment of F*128 entries ->
    ([16, F*8] i16 stripe, [128, F] f32 slots)."""
    n = F * P
    ii = np.zeros(n, np.int64)
    ss = np.full(n, -1.0, np.float32)
    ii[:len(idx)] = idx
    ss[:len(slot)] = slot
    stripe = ii.reshape(F * 8, 16).T.astype(np.int16)      # j%16, j//16
    slots = ss.reshape(F, 128).T.copy()                    # j%128, j//128
    return stripe, slots


def _preprocess(V, E):
    V = np.asarray(V).astype(np.int64)
    E = np.asarray(E).astype(np.int64)

    # ---- phase A: sort by E, window by 128 edges, lo/hi split by V ----
    oA = np.argsort(E, kind='stable')
    Es, Vs = E[oA], V[oA]
    bndA = np.searchsorted(Es, np.arange(0, NCORES * ESH + 1, P))
    # per (core, window): lo/hi entry lists
    a_ent = [[None] * W_A for _ in range(NCORES)]
    FLO = np.zeros(W_A, np.int64)
    FHI = np.zeros(W_A, np.int64)
    for c in range(NCORES):
        for w in range(W_A):
            wi = c * W_A + w
            lo, hi = bndA[wi], bndA[wi + 1]
            v = Vs[lo:hi]
            s = (Es[lo:hi] - wi * P).astype(np.float32)
            m = v < LO
            a_ent[c][w] = (v[m], s[m], v[~m] - LO, s[~m])
            FLO[w] = max(FLO[w], (m.sum() + P - 1) // P)
            FHI[w] = max(FHI[w], ((~m).sum() + P - 1) // P)
    FLO = np.maximum(FLO, 1)
    FHI = np.maximum(FHI, 1)
    FA = FLO + FHI
    CA = int(FA.sum()) * 8                    # idx cols per stripe
    SA = int(FA.sum())                        # slot cols
    a_idx = np.zeros((NCORES, 16, CA), np.int16)
    a_slot = np.full((NCORES, 128, SA), -1.0, np.float32)
    for c in range(NCORES):
        co = so = 0
        for w in range(W_A):
            vl, sl, vh, sh = a_ent[c][w]
            st, sv = _pack_seq(vl, sl, int(FLO[w]))
            a_idx[c, :, co:co + FLO[w] * 8] = st
            a_slot[c, :, so:so + FLO[w]] = sv
            co += FLO[w] * 8
            so += int(FLO[w])
            st, sv = _pack_seq(vh, sh, int(FHI[w]))
            a_idx[c, :, co:co + FHI[w] * 8] = st
            a_slot[c, :, so:so + FHI[w]] = sv
            co += FHI[w] * 8
            so += int(FHI[w])

    cntE = np.bincount(E, minlength=N_EDGE).astype(np.float64)
    recipE = (1.0 / np.maximum(cntE, 1.0)).astype(np.float32)
    er = np.arange(NCORES * ESH)
    a_recip = np.where(er < N_EDGE, recipE[np.minimum(er, N_EDGE - 1)],
                       0.0).astype(np.float32).reshape(NCORES, W_A, P)

    # ---- phase B: sort by V, window by 128 nodes ----
    oB = np.argsort(V, kind='stable')
    Vs2, Es2 = V[oB], E[oB]
    rows2 = _row_of_edge(Es2)
    bndB = np.searchsorted(Vs2, np.arange(0, NCORES * NSH + 1, P))
    b_ent = [[None] * W_B for _ in range(NCORES)]
    FB = np.zeros(W_B, np.int64)
    for c in range(NCORES):
        for w in range(W_B):
            wi = c * W_B + w
            lo, hi = bndB[wi], bndB[wi + 1]
            b_ent[c][w] = (rows2[lo:hi],
                           (Vs2[lo:hi] - wi * P).astype(np.float32))
            FB[w] = max(FB[w], (hi - lo + P - 1) // P)
    FB = np.maximum(FB, 1)
    CB = int(FB.sum()) * 8
    SB = int(FB.sum())
    b_idx = np.zeros((NCORES, 16, CB), np.int16)
    b_slot = np.full((NCORES, 128, SB), -1.0, np.float32)
    for c in range(NCORES):
        co = so = 0
        for w in range(W_B):
            rr, ssl = b_ent[c][w]
            st, sv = _pack_seq(rr, ssl, int(FB[w]))
            b_idx[c, :, co:co + FB[w] * 8] = st
            b_slot[c, :, so:so + FB[w]] = sv
            co += FB[w] * 8
            so += int(FB[w])

    cntV = np.bincount(V, minlength=N_NODE).astype(np.float64)
    recipV = (1.0 / np.maximum(cntV, 1.0)).astype(np.float32)
    nr = np.arange(NCORES * NSH)
    b_recip = np.where(nr < N_NODE, recipV[np.minimum(nr, N_NODE - 1)],
                       0.0).astype(np.float32).reshape(NCORES, W_B, P)

    return dict(FLO=tuple(int(x) for x in FLO), FHI=tuple(int(x) for x in FHI),
                FB=tuple(int(x) for x in FB),
                a_idx=np.tile(a_idx, (1, 8, 1)), a_slot=a_slot,
                a_recip=a_recip,
                b_idx=np.tile(b_idx, (1, 8, 1)), b_slot=b_slot,
                b_recip=b_recip)


def _emulate(pp, inp_f32, weight, bias):
    """Numpy emulation of the exact device program."""
    FLO, FHI, FB = pp['FLO'], pp['FHI'], pp['FB']
    y_full = np.zeros((NCORES * ESH, D), np.float32)
    out = np.zeros((NCORES * NSH, D), np.float32)
    for c in range(NCORES):
        so = 0
        for w in range(W_A):
            F = FLO[w] + FHI[w]
            idx = pp['a_idx'][c, :16, so * 8:(so + F) * 8].T.reshape(-1)
            slot = pp['a_slot'][c, :, so:so + F].T.reshape(-1)
            # lo rows gathered from X[:LO], hi rows from X[LO:]
            g = np.zeros((F * P, D), np.float32)
            nlo = FLO[w] * P
            g[:nlo] = inp_f32[idx[:nlo].astype(np.int64)]
            g[nlo:] = inp_f32[idx[nlo:].astype(np.int64) + LO]
            acc = np.zeros((P, D), np.float32)
            for j in range(F * P):
                if slot[j] >= 0:
                    acc[int(slot[j])] += g[j]
            r0 = _row_of_edge(c * ESH + w * P)
            y_full[r0:r0 + P] = acc * pp['a_recip'][c, w][:, None]
            so += F
    for c in range(NCORES):
        so = 0
        for w in range(W_B):
            F = FB[w]
            idx = pp['b_idx'][c, :16, so * 8:(so + F) * 8].T.reshape(-1)
            slot = pp['b_slot'][c, :, so:so + F].T.reshape(-1)
            g = y_full[idx.astype(np.int64)]
            acc = np.zeros((P, D), np.float32)
            for j in range(F * P):
                if slot[j] >= 0:
                    acc[int(slot[j])] += g[j]
            res = (acc @ weight) * pp['b_recip'][c, w][:, None] + bias[None, :]
            out[c * NSH + w * P:c * NSH + (w + 1) * P] = res
            so += F
    rows = []
    for c in range(NCORES):
        n = min(NSH, N_NODE - c * NSH)
        rows.append(out[c * NSH:c * NSH + n])
    return np.concatenate(rows, 0)


def _build_program(FLO, FHI, FB):
    import concourse.bacc as bacc
    import concourse.bass as bass
    import concourse.tile as tile
    from concourse import mybir

    f32 = mybir.dt.float32
    bf16 = mybir.dt.bfloat16
    i16 = mybir.dt.int16

    FA = [l + h for l, h in zip(FLO, FHI)]
    CA = sum(FA) * 8
    SA = sum(FA)
    CB = sum(FB) * 8
    SB = sum(FB)
    FAmax = max(FA)
    FBmax = max(FB)

    nc = bacc.Bacc(None, target_bir_lowering=False, debug=False,
                   num_swdge_queues=NQ,
                   dynamic_dma_scratch_size=int(os.environ.get('KERNEL_SCR', '16384')))
    inp = nc.dram_tensor("input", [N_NODE, D], bf16, kind="ExternalInput")
    wgt = nc.dram_tensor("wgt", [D, D], bf16, kind="ExternalInput")
    bias_bc = nc.dram_tensor("bias_bc", [P, D], f32, kind="ExternalInput")
    iota_in = nc.dram_tensor("iota_in", [P, P], f32, kind="ExternalInput")
    a_idx = nc.dram_tensor("a_idx", [128, CA], i16, kind="ExternalInput")
    a_slot = nc.dram_tensor("a_slot", [128, SA], f32, kind="ExternalInput")
    a_recip = nc.dram_tensor("a_recip", [W_A, P], f32, kind="ExternalInput")
    b_idx = nc.dram_tensor("b_idx", [128, CB], i16, kind="ExternalInput")
    b_slot = nc.dram_tensor("b_slot", [128, SB], f32, kind="ExternalInput")
    b_recip = nc.dram_tensor("b_recip", [W_B, P], f32, kind="ExternalInput")
    out = nc.dram_tensor("out", [NSH, D], f32, kind="ExternalOutput")

    eq = mybir.AluOpType.is_equal
    mul = mybir.AluOpType.mult
    add = mybir.AluOpType.add

    qn = [0]

    def gather_seg(dst3, table, idx_t, c0, F):
        """Chunked dma_gather of F*128 rows into dst3 [128, F, 128]."""
        n = F * P
        for r0 in range(0, n, CHR):
            r1 = min(r0 + CHR, n)
            nc.gpsimd.dma_gather(
                dst3[:, r0 // P:r1 // P, :], table,
                idx_t[:, c0 + r0 // 16:c0 + r1 // 16],
                r1 - r0, r1 - r0, P, queue_num=qn[0] % NQ)
            qn[0] += 1

    with tile.TileContext(nc) as tc:
        with tc.tile_pool(name="const", bufs=1) as cpool, \
             tc.tile_pool(name="gat", bufs=int(os.environ.get('KERNEL_GBUFS', '3'))) as gpool, \
             tc.tile_pool(name="one", bufs=int(os.environ.get('KERNEL_OBUFS', '3'))) as opool, \
             tc.tile_pool(name="res", bufs=4) as respool, \
             tc.tile_pool(name="pacc", bufs=int(os.environ.get('KERNEL_PBUFS', '2')), space="PSUM") as pacc, \
             tc.tile_pool(name="pres", bufs=2, space="PSUM") as pres, \
             tc.tile_pool(name="dram", bufs=1, space="DRAM") as dpool:

            iota_t = cpool.tile([P, P], f32)
            nc.sync.dma_start(out=iota_t[:], in_=iota_in[:])
            wgt_t = cpool.tile([D, D], bf16)
            nc.sync.dma_start(out=wgt_t[:], in_=wgt[:])
            bias_t = cpool.tile([P, D], f32)
            nc.sync.dma_start(out=bias_t[:], in_=bias_bc[:])
            a_idx_t = cpool.tile([128, CA], i16)
            nc.sync.dma_start(out=a_idx_t[:], in_=a_idx[:])
            a_slot_t = cpool.tile([128, SA], f32)
            nc.sync.dma_start(out=a_slot_t[:], in_=a_slot[:])
            b_idx_t = cpool.tile([128, CB], i16)
            nc.sync.dma_start(out=b_idx_t[:], in_=b_idx[:])
            b_slot_t = cpool.tile([128, SB], f32)
            nc.sync.dma_start(out=b_slot_t[:], in_=b_slot[:])
            a_recip_t = cpool.tile([P, W_A], f32)
            nc.sync.dma_start(out=a_recip_t[:], in_=a_recip[:].transpose([1, 0]))
            b_recip_t = cpool.tile([P, W_B], f32)
            nc.sync.dma_start(out=b_recip_t[:], in_=b_recip[:].transpose([1, 0]))

            y_shard = dpool.tile([ESH, D], bf16)
            y_full = dpool.tile([NCORES * ESH, D], bf16)
            y_ch = [dpool.tile([NCORES * CH_E, D], bf16, addr_space="Shared",
                               name=f"y_ch{k}") for k in range(NCH)]

            # ---------------- Phase A ----------------
            so = 0
            for w in range(W_A):
                F = FA[w]
                ga = gpool.tile([P, FAmax, P], bf16, name="ga", tag="ga")
                gather_seg(ga[:, :FLO[w], :], inp[:LO, :], a_idx_t,
                           so * 8, FLO[w])
                gather_seg(ga[:, FLO[w]:F, :], inp[LO:, :], a_idx_t,
                           (so + FLO[w]) * 8, FHI[w])
                o = opool.tile([P, FAmax, P], bf16, name="oa", tag="oa")
                nc.vector.tensor_tensor(
                    out=o[:, :F, :],
                    in0=iota_t[:].unsqueeze(1).broadcast_to([P, F, P]),
                    in1=a_slot_t[:, so:so + F].unsqueeze(2)
                        .broadcast_to([P, F, P]),
                    op=eq)
                acc = pacc.tile([P, D], f32)
                for f in range(F):
                    nc.tensor.matmul(acc[:], lhsT=o[:, f, :], rhs=ga[:, f, :],
                                     start=(f == 0), stop=(f == F - 1))
                yw = respool.tile([P, D], bf16, name="yw", tag="yw")
                nc.vector.tensor_tensor(
                    out=yw[:], in0=acc[:],
                    in1=a_recip_t[:, w:w + 1].to_broadcast([P, D]), op=mul)
                nc.sync.dma_start(out=y_shard[w * P:(w + 1) * P, :], in_=yw[:])
                so += F
                if (w + 1) % (W_A // NCH) == 0:
                    k = w // (W_A // NCH)
                    nc.gpsimd.collective_compute(
                        "AllGather", mybir.AluOpType.bypass,
                        replica_groups=[list(range(NCORES))],
                        ins=[y_shard[k * CH_E:(k + 1) * CH_E, :]],
                        outs=[y_ch[k].opt()])
                    nc.sync.dma_start(
                        out=y_full[k * NCORES * CH_E:
                                   (k + 1) * NCORES * CH_E, :],
                        in_=y_ch[k][:])

            # ---------------- Phase B ----------------
            so = 0
            for w in range(W_B):
                F = FB[w]
                gb = gpool.tile([P, FBmax, P], bf16, name="gb", tag="gb")
                gather_seg(gb[:, :F, :], y_full[:], b_idx_t, so * 8, F)
                o = opool.tile([P, FBmax, P], bf16, name="ob", tag="ob")
                nc.vector.tensor_tensor(
                    out=o[:, :F, :],
                    in0=iota_t[:].unsqueeze(1).broadcast_to([P, F, P]),
                    in1=b_slot_t[:, so:so + F].unsqueeze(2)
                        .broadcast_to([P, F, P]),
                    op=eq)
                acc2 = pacc.tile([P, D], f32, name="acc2")
                for f in range(F):
                    nc.tensor.matmul(acc2[:], lhsT=gb[:, f, :], rhs=o[:, f, :],
                                     start=(f == 0), stop=(f == F - 1))
                zt = respool.tile([P, D], bf16, name="zt", tag="zt")
                nc.vector.tensor_copy(out=zt[:], in_=acc2[:])
                res_p = pres.tile([P, D], f32, name="res_p")
                nc.tensor.matmul(res_p[:], lhsT=zt[:], rhs=wgt_t[:],
                                 start=True, stop=True)
                tmp = respool.tile([P, D], f32, name="tmpb")
                nc.vector.tensor_tensor(
                    out=tmp[:], in0=res_p[:],
                    in1=b_recip_t[:, w:w + 1].to_broadcast([P, D]), op=mul)
                res = respool.tile([P, D], f32, name="resb")
                nc.vector.tensor_tensor(out=res[:], in0=tmp[:], in1=bias_t[:],
                                        op=add)
                nc.sync.dma_start(out=out[w * P:(w + 1) * P, :], in_=res[:])
                so += F

    nc.compile()
    return nc


def make_in_maps(pp, inp, wgt, b):
    import ml_dtypes
    bf = ml_dtypes.bfloat16
    iota_np = np.tile(np.arange(P, dtype=np.float32), (P, 1))
    bias_bc = np.tile(b[None, :], (P, 1)).astype(np.float32)
    in_maps = []
    for c in range(NCORES):
        in_maps.append(dict(
            input=inp.astype(bf), wgt=wgt.astype(bf),
            bias_bc=bias_bc, iota_in=iota_np,
            a_idx=pp['a_idx'][c], a_slot=pp['a_slot'][c],
            a_recip=pp['a_recip'][c],
            b_idx=pp['b_idx'][c], b_slot=pp['b_slot'][c],
            b_recip=pp['b_recip'][c]))
    return in_maps


def kernel(input, weight, bias, V, E, num_edges):
    global LAST_RESULTS
    inp = np.ascontiguousarray(np.asarray(input), dtype=np.float32)
    wgt = np.ascontiguousarray(np.asarray(weight), dtype=np.float32)
    b = np.asarray(bias).astype(np.float32)
    pp = _preprocess(V, E)

    if os.environ.get('KERNEL_EMULATE'):
        return _emulate(pp, inp, wgt, b)

    from concourse.bass_utils import run_bass_kernel_spmd

    key = (pp['FLO'], pp['FHI'], pp['FB'])
    if key not in _PROG_CACHE:
        _PROG_CACHE[key] = _build_program(*key)
    nc = _PROG_CACHE[key]

    in_maps = make_in_maps(pp, inp, wgt, b)
    trace = bool(os.environ.get('KERNEL_TRACE'))
    res = run_bass_kernel_spmd(nc, in_maps, list(range(NCORES)), trace=trace)
    LAST_RESULTS = res
    rows = []
    for c in range(NCORES):
        n = min(NSH, N_NODE - c * NSH)
        rows.append(res.results[c]['out'][:n])
    return np.concatenate(rows, 0).astype(np.float32)
